# revision 57
# baseline (speedup 1.0000x reference)
"""Block-sparse causal MHA with ALiBi on 8 Trainium2 NeuronCores.

Problem: B=2, T=2048, D=1024, H=16, hd=64. Mask analysis: with
BLOCK_SPARSE_SIZE=128 and WINDOW_SIZE=256, each 128-query block attends
causally to itself and fully to the previous 128-block (window constraint is
implied: max dist = 255). Sharding: core i = (batch i//4, seq chunk i%4 of 512
queries). Each core computes QKV projections from a host-pre-transposed x
slice (640 kv rows = 512 own + 128 prev), block-sparse attention for all 16
heads, and the output projection for its 512 rows. Host assembles the full
(out, attn) tuple; attn is zero outside the block-sparse bands.

Softmax is computed without a max-reduction: scores are shifted by the
compile-time row bound C_h[q] = slope_h*(128+qi) + 20 (>= row max up to the
bounded QK term), applied via the ACT exp bias. The ALiBi+mask table B_h
(exactly representable in bf16: integer dists times power-of-2 slopes) is
accumulated into the score PSUM via an identity-weighted matmul.
"""

import math
import sys

sys.path.insert(0, "/opt/trn_rl_repo")

import numpy as np

import concourse.bass as bass
import concourse.mybir as mybir
import concourse.tile as tile
from concourse import bacc, bass_utils
from concourse.masks import make_identity

# ---- problem constants (hardcoded; kernel.py must be self-contained) ----
B = 2
T = 2048
DM = 1024
H = 16
HD = 64
NCORES = 8
SCHUNK = 4          # seq chunks per batch
TQ = T // SCHUNK    # queries per core (512)
PREV = 128          # previous block carried for K/V
TKV = TQ + PREV     # kv rows per core (640)
QB = TQ // 128      # 128-query blocks per core (4)
KB = TKV // 128     # 128-kv blocks per core (5)
NK = DM // 128      # contraction chunks (8)
MASKVAL = -1e9
SCALE = 1.0 / math.sqrt(HD)
CPAD = 20.0         # safety margin over the QK-score magnitude

F32 = mybir.dt.float32
BF16 = mybir.dt.bfloat16
XW_DT = BF16        # dtype for x/W matmul operands
AV_DT = BF16        # dtype for P^T and V in the attn@V matmul
P_DT = F32          # probs staging dtype

_SLOPES = [2.0 ** (-8.0 * (h + 1) / H) for h in range(H)]


def build_program():
    nc = bacc.Bacc("TRN2", target_bir_lowering=False, debug=False)

    xt_d = nc.dram_tensor("xt", (DM, TKV), XW_DT, kind="ExternalInput")
    wq_d = nc.dram_tensor("wq", (DM, DM), XW_DT, kind="ExternalInput")
    wk_d = nc.dram_tensor("wk", (DM, DM), XW_DT, kind="ExternalInput")
    wv_d = nc.dram_tensor("wv", (DM, DM), XW_DT, kind="ExternalInput")
    wo_d = nc.dram_tensor("wo", (DM, DM), XW_DT, kind="ExternalInput")
    bqs_d = nc.dram_tensor("bqs", (DM,), F32, kind="ExternalInput")  # 0.125*b_q
    bk_d = nc.dram_tensor("bk", (DM,), F32, kind="ExternalInput")
    co_d = nc.dram_tensor("co", (DM,), F32, kind="ExternalInput")   # b_v@W_o+b_o
    cb_d = nc.dram_tensor("cb", (128, 2 * H), F32, kind="ExternalInput")  # -C_h[q]
    # ALiBi+mask tables: hi = bf16(slope*dist(+mask)), lo = bf16 residual of
    # the unmasked bias (needed only for the 8 irrational-slope even heads)
    bm_d = nc.dram_tensor("bm", (H, 128, 256), BF16, kind="ExternalInput")
    bf_d = nc.dram_tensor("bf", (H, 128, 256), BF16, kind="ExternalInput")
    blo_d = nc.dram_tensor("blo", (H // 2, 128, 256), BF16, kind="ExternalInput")

    y_d = nc.dram_tensor("y", (TQ, DM), F32, kind="ExternalOutput")
    probs_d = nc.dram_tensor("probs", (QB * H, 128, 256), P_DT,
                             kind="ExternalOutput")

    with tile.TileContext(nc) as tc:
        with (
            tc.tile_pool(name="persist", bufs=1) as persist,
            tc.tile_pool(name="attn_sb", bufs=6) as attn_sb,
            tc.tile_pool(name="pstage", bufs=2) as pstage_pool,
            tc.tile_pool(name="small", bufs=8) as small,
            tc.tile_pool(name="proj_ps", bufs=2, space="PSUM") as proj_ps,
            tc.tile_pool(name="s_ps", bufs=2, space="PSUM") as s_ps,
            tc.tile_pool(name="pt_ps", bufs=2, space="PSUM") as pt_ps,
            tc.tile_pool(name="ot_ps", bufs=2, space="PSUM") as ot_ps,
        ):
            # ---- PE warmup: matmuls on a zeroed tile keep the HAM clock
            # gate open while the input DMAs stream in ----
            warm = persist.tile([128, 512], BF16, tag="warm", name="warm")
            nc.vector.memset(warm, 0.0)
            for w in range(40):
                wps = proj_ps.tile([128, TQ], F32, tag="proj", name="wps")
                nc.tensor.matmul(wps, warm[:, 0:128], warm, start=True, stop=True)

            # ---- bulk input loads: one DMA per tensor, split across the two
            # HWDGE queues (SP and ACT) ----
            xt = persist.tile([128, NK * TKV], XW_DT, tag="xt", name="xt")

            def load_w(handle, qeng, name):
                t = persist.tile([128, NK * DM], XW_DT, tag=name, name=name)
                qeng.dma_start(
                    out=t.rearrange("p (k j) -> p k j", j=DM),
                    in_=handle[:, :].rearrange("(k p) j -> p k j", p=128))
                return t

            # xt and wq land first, split into k-halves interleaved across
            # the two HWDGE queues so the first Qt matmuls start early;
            # wk/wv follow, wo (needed last) trails
            wq = persist.tile([128, NK * DM], XW_DT, tag="wq_sb", name="wq_sb")
            for kh in range(2):
                k0, k1 = kh * (NK // 2), (kh + 1) * (NK // 2)
                nc.sync.dma_start(
                    out=xt.rearrange("p (k j) -> p k j", j=TKV)[:, k0:k1, :],
                    in_=xt_d[k0 * 128:k1 * 128, :]
                    .rearrange("(k p) j -> p k j", p=128))
                nc.scalar.dma_start(
                    out=wq.rearrange("p (k j) -> p k j", j=DM)[:, k0:k1, :],
                    in_=wq_d[k0 * 128:k1 * 128, :]
                    .rearrange("(k p) j -> p k j", p=128))
            wk = load_w(wk_d, nc.sync, "wk_sb")
            wv = load_w(wv_d, nc.sync, "wv_sb")
            wo = load_w(wo_d, nc.sync, "wo_sb")

            bq_sb = persist.tile([128, NK], F32, tag="bq_sb", name="bq_sb")
            bk_sb = persist.tile([128, NK], F32, tag="bk_sb", name="bk_sb")
            nc.gpsimd.dma_start(out=bq_sb,
                                in_=bqs_d[:].rearrange("(c p) -> p c", p=128))
            nc.gpsimd.dma_start(out=bk_sb,
                                in_=bk_d[:].rearrange("(c p) -> p c", p=128))

            cb_sb = persist.tile([128, 2 * H], F32, tag="cb_sb", name="cb_sb")
            nc.gpsimd.dma_start(out=cb_sb, in_=cb_d[:, :])

            co_sb = persist.tile([128, DM], F32, tag="co_sb", name="co_sb")
            co_ap = co_d[:]
            co_bcast = bass.AP(
                tensor=co_ap.tensor, offset=co_ap.offset,
                ap=[[0, 128]] + list(co_ap.ap),
            )
            nc.gpsimd.dma_start(out=co_sb, in_=co_bcast)

            bm_sb = persist.tile([128, H * 256], BF16, tag="bm_sb", name="bm_sb")
            bf_sb = persist.tile([128, H * 256], BF16, tag="bf_sb", name="bf_sb")
            blo_sb = persist.tile([128, (H // 2) * 256], BF16, tag="blo_sb",
                                  name="blo_sb")
            nc.sync.dma_start(
                out=bm_sb.rearrange("p (h j) -> p h j", j=256),
                in_=bm_d[:, :, :].rearrange("h p j -> p h j"))
            nc.sync.dma_start(
                out=bf_sb.rearrange("p (h j) -> p h j", j=256),
                in_=bf_d[:, :, :].rearrange("h p j -> p h j"))
            nc.sync.dma_start(
                out=blo_sb.rearrange("p (h j) -> p h j", j=256),
                in_=blo_d[:, :, :].rearrange("h p j -> p h j"))

            ident = persist.tile([128, 128], F32, tag="ident", name="ident")
            make_identity(nc, ident)
            ident_bf = persist.tile([128, 128], BF16, tag="ident_bf",
                                    name="ident_bf")
            make_identity(nc, ident_bf)


            # ---- projections ----
            # Qt[c] = ((x @ W_q)*0.125 + 0.125*b_q)^T col-block: [128, 512]
            qt = [persist.tile([128, TQ], XW_DT, tag=f"qt{c}", name=f"qt{c}")
                  for c in range(NK)]
            kt = [persist.tile([128, TKV], XW_DT, tag=f"kt{c}", name=f"kt{c}")
                  for c in range(NK)]
            for c in range(NK):
                ps = proj_ps.tile([128, TQ], F32, tag="proj", name="qt_ps")
                for k in range(NK):
                    nc.tensor.matmul(
                        ps,
                        wq[:, k * DM + c * 128:k * DM + (c + 1) * 128],
                        xt[:, k * TKV + PREV:(k + 1) * TKV],
                        start=(k == 0), stop=(k == NK - 1))
                if c % 2 == 0:
                    nc.vector.tensor_scalar(
                        out=qt[c], in0=ps, scalar1=SCALE,
                        scalar2=bq_sb[:, c:c + 1],
                        op0=mybir.AluOpType.mult, op1=mybir.AluOpType.add)
                else:
                    nc.scalar.activation(
                        out=qt[c], in_=ps,
                        func=mybir.ActivationFunctionType.Identity,
                        bias=bq_sb[:, c:c + 1], scale=SCALE)

                for (n0, n1) in ((0, TQ), (TQ, TKV)):
                    ps2 = proj_ps.tile([128, TQ], F32, tag="proj", name="kt_ps")
                    for k in range(NK):
                        nc.tensor.matmul(
                            ps2[:, 0:n1 - n0],
                            wk[:, k * DM + c * 128:k * DM + (c + 1) * 128],
                            xt[:, k * TKV + n0:k * TKV + n1],
                            start=(k == 0), stop=(k == NK - 1))
                    if c % 2 == 0:
                        nc.vector.tensor_scalar(
                            out=kt[c][:, n0:n1], in0=ps2[:, 0:n1 - n0],
                            scalar1=1.0, scalar2=bk_sb[:, c:c + 1],
                            op0=mybir.AluOpType.mult, op1=mybir.AluOpType.add)
                    else:
                        nc.scalar.activation(
                            out=kt[c][:, n0:n1], in_=ps2[:, 0:n1 - n0],
                            func=mybir.ActivationFunctionType.Identity,
                            bias=bk_sb[:, c:c + 1], scale=1.0)

            # V natural layout: 5 tiles [128 kv rows, 1024 cols]
            vt = [persist.tile([128, DM], AV_DT, tag=f"vt{t}", name=f"vt{t}")
                  for t in range(KB)]

            def emit_v(t):
                for half in range(2):
                    ps = proj_ps.tile([128, TQ], F32, tag="proj", name="v_ps")
                    for k in range(NK):
                        nc.tensor.matmul(
                            ps,
                            xt[:, k * TKV + t * 128:k * TKV + (t + 1) * 128],
                            wv[:, k * DM + half * 512:k * DM + (half + 1) * 512],
                            start=(k == 0), stop=(k == NK - 1))
                    nc.vector.tensor_copy(
                        out=vt[t][:, half * 512:(half + 1) * 512], in_=ps)

            with tc.high_priority(offset=-1000):
                for t in range(KB):
                    emit_v(t)

            # ---- attention + output projection, per query block ----
            ot_sb = [[attn_sb.tile([128, 128], AV_DT, tag=f"ot{qb}_{p}",
                                   name=f"ot{qb}_{p}", bufs=1)
                      for p in range(H // 2)] for qb in range(QB)]
            for qb in range(QB):
                pst = pstage_pool.tile([128, H * 256], P_DT, tag="pst", name="pst")
                for h in range(H):
                    c, hi = h // 2, h % 2
                    # scores S = Qt_h^T @ Kt_h + B_h  [128q, 256k] in PSUM
                    s_psum = s_ps.tile([128, 256], F32, tag="s", name="s_psum")
                    nc.tensor.matmul(
                        s_psum,
                        qt[c][hi * 64:(hi + 1) * 64, qb * 128:(qb + 1) * 128],
                        kt[c][hi * 64:(hi + 1) * 64, qb * 128:qb * 128 + 256],
                        start=True, stop=False)
                    btab = bf_sb if qb == 0 else bm_sb
                    even = (h % 2 == 0)
                    nc.tensor.matmul(
                        s_psum, ident_bf, btab[:, h * 256:(h + 1) * 256],
                        start=False, stop=not even)
                    if even:
                        nc.tensor.matmul(
                            s_psum, ident_bf,
                            blo_sb[:, (h // 2) * 256:(h // 2 + 1) * 256],
                            start=False, stop=True)
                    # p_raw = exp(S + B - C_h), rowsum accumulated per query
                    p_raw = attn_sb.tile([128, 256], F32, tag="p_raw",
                                         name="p_raw")
                    sumexp = small.tile([128, 1], F32, tag="sumexp", name="sumexp")
                    nc.scalar.activation(
                        out=p_raw, in_=s_psum,
                        func=mybir.ActivationFunctionType.Exp,
                        bias=cb_sb[:, (H + h if qb == 0 else h):
                                    (H + h if qb == 0 else h) + 1],
                        scale=1.0, accum_out=sumexp)
                    rinv = small.tile([128, 1], F32, tag="rinv", name="rinv")
                    nc.vector.reciprocal(rinv, sumexp)
                    # normalized probs (also the DMA staging slice), on GPSIMD
                    pn = pst[:, h * 256:(h + 1) * 256]
                    nc.gpsimd.tensor_scalar_mul(pn, p_raw, rinv)

                    # bf16 copy of normalized probs feeds the transposes
                    pnb = attn_sb.tile([128, 256], BF16, tag="pnb", name="pnb")
                    nc.vector.tensor_copy(out=pnb, in_=pn)
                    ptp = pt_ps.tile([128, 256], BF16, tag="pt", name="ptp")
                    nc.tensor.transpose(ptp[:, 0:128], pnb[:, 0:128], ident_bf)
                    nc.tensor.transpose(ptp[:, 128:256], pnb[:, 128:256],
                                        ident_bf)
                    pt_sb = attn_sb.tile([128, 256], AV_DT, tag="pt_sb",
                                         name="pt_sb")
                    nc.vector.tensor_copy(out=pt_sb, in_=ptp)

                    # attn-out^T [64, 128q], accumulated over 2 key chunks
                    otp = ot_ps.tile([64, 128], F32, tag="ot", name="otp")
                    for kc in range(2):
                        nc.tensor.matmul(
                            otp, vt[qb + kc][:, h * 64:(h + 1) * 64],
                            pt_sb[:, kc * 128:(kc + 1) * 128],
                            start=(kc == 0), stop=(kc == 1))
                    if h % 2 == 0:
                        nc.vector.tensor_copy(
                            out=ot_sb[qb][c][hi * 64:(hi + 1) * 64, :], in_=otp)
                    else:
                        nc.scalar.copy(
                            out=ot_sb[qb][c][hi * 64:(hi + 1) * 64, :], in_=otp)
                    if qb == QB - 1 and h in (3, 7, 11):
                        h0 = h - 3
                        nc.sync.dma_start(
                            out=probs_d[qb * H + h0:qb * H + h + 1, :, :]
                            .rearrange("h p j -> p h j"),
                            in_=pst[:, h0 * 256:(h + 1) * 256]
                            .rearrange("p (h j) -> p h j", j=256))



                if qb == QB - 1:
                    nc.sync.dma_start(
                        out=probs_d[qb * H + 12:(qb + 1) * H, :, :]
                        .rearrange("h p j -> p h j"),
                        in_=pst[:, 12 * 256:].rearrange("p (h j) -> p h j", j=256))
                else:
                    nc.sync.dma_start(
                        out=probs_d[qb * H:(qb + 1) * H, :, :]
                        .rearrange("h p j -> p h j"),
                        in_=pst.rearrange("p (h j) -> p h j", j=256))

                # output projection for this query block
                y_sb = attn_sb.tile([128, DM], F32, tag="y_sb", name="y_sb", bufs=2)
                wo_prio = tc.high_priority(offset=-1000 if qb < QB - 1 else 0)
                with wo_prio:
                 for half in range(2):
                    yp = proj_ps.tile([128, TQ], F32, tag="proj", name="yp")
                    for p in range(NK):
                        nc.tensor.matmul(
                            yp, ot_sb[qb][p],
                            wo[:, p * DM + half * 512:p * DM + (half + 1) * 512],
                            start=(p == 0), stop=(p == NK - 1))
                    nc.vector.tensor_add(
                        y_sb[:, half * 512:(half + 1) * 512], yp,
                        co_sb[:, half * 512:(half + 1) * 512])
                    nc.sync.dma_start(
                        out=y_d[qb * 128:(qb + 1) * 128,
                                half * 512:(half + 1) * 512],
                        in_=y_sb[:, half * 512:(half + 1) * 512])

    nc.compile()
    return nc


_NC = None


def _get_program():
    global _NC
    if _NC is None:
        _NC = build_program()
    return _NC


def _np_dt(dt):
    return mybir.dt.np(dt)


def _host_tables():
    import ml_dtypes
    bf16 = ml_dtypes.bfloat16
    qi = np.arange(128, dtype=np.float32)[:, None]
    kj = np.arange(256, dtype=np.float32)[None, :]
    left = 128.0 + qi - kj
    right = qi - (kj - 128.0)
    dist = np.where(kj < 128, left, right).astype(np.float32)  # unmasked
    dm = np.where(kj < 128, left,
                  np.where(right >= 0, right, MASKVAL)).astype(np.float32)
    dmf = dm.copy()
    dmf[:, :128] = MASKVAL
    slopes = np.asarray(_SLOPES, np.float32)
    bm = np.stack([(s * dm) for s in slopes]).astype(bf16)          # (16,128,256)
    bfv = np.stack([(s * dmf) for s in slopes]).astype(bf16)
    blo = np.stack([
        (slopes[h] * dist) - bm[h].astype(np.float32)
        for h in range(0, H, 2)
    ]).astype(np.float32)
    # residual only where unmasked (masked entries are hugely negative anyway)
    blo[:, dm == np.float32(MASKVAL)] = 0.0
    blo = blo.astype(bf16)
    cb = -(slopes[None, :] * (128.0 + qi) + CPAD).astype(np.float32)
    # qb==0 variant: on first-chunk cores the prev block is masked, so the
    # row max bound is slope*qi, not slope*(128+qi)
    cb0 = -(slopes[None, :] * qi + CPAD).astype(np.float32)
    cb_first = np.concatenate([cb, cb0], axis=1)
    cb_rest = np.concatenate([cb, cb], axis=1)
    return bm, bfv, blo, (np.ascontiguousarray(cb_first, dtype=np.float32),
                          np.ascontiguousarray(cb_rest, dtype=np.float32))


def kernel(x, W_q, b_q, W_k, b_k, W_v, b_v, W_o, b_o):
    x = np.asarray(x, np.float32)
    xw_np = _np_dt(XW_DT)

    nc = _get_program()

    wq_c = np.asarray(W_q, np.float32).astype(xw_np)
    wk_c = np.asarray(W_k, np.float32).astype(xw_np)
    wv_c = np.asarray(W_v, np.float32).astype(xw_np)
    wo_c = np.asarray(W_o, np.float32).astype(xw_np)
    bqs = (np.asarray(b_q, np.float32) * SCALE).astype(np.float32)
    bk = np.asarray(b_k, np.float32)
    co = (np.asarray(b_v, np.float64) @ np.asarray(W_o, np.float64)
          + np.asarray(b_o, np.float64)).astype(np.float32)
    bm, bfv, blo, (cb_first, cb_rest) = _host_tables()
    bf_first = bfv

    in_maps = []
    for i in range(NCORES):
        b, s = i // SCHUNK, i % SCHUNK
        if s == 0:
            xs = np.concatenate(
                [np.zeros((PREV, DM), np.float32), x[b, 0:TQ]], axis=0)
        else:
            xs = x[b, s * TQ - PREV:(s + 1) * TQ]
        xT = np.ascontiguousarray(xs.T).astype(xw_np)
        in_maps.append({
            "xt": xT, "wq": wq_c, "wk": wk_c, "wv": wv_c, "wo": wo_c,
            "bqs": bqs, "bk": bk, "co": co,
            "cb": cb_first if s == 0 else cb_rest,
            "bm": bm, "bf": bf_first if s == 0 else bm, "blo": blo,
        })

    res = bass_utils.run_bass_kernel_spmd(nc, in_maps,
                                          core_ids=list(range(NCORES)))

    out = np.empty((B, T, DM), np.float32)
    attn = np.zeros((B, H, T, T), np.float32)
    for i in range(NCORES):
        b, s = i // SCHUNK, i % SCHUNK
        out[b, s * TQ:(s + 1) * TQ] = res.results[i]["y"]
        pr = res.results[i]["probs"].astype(np.float32).reshape(QB, H, 128, 256)
        for qb in range(QB):
            g = SCHUNK * s + qb
            q0 = 128 * g
            for h in range(H):
                if g == 0:
                    attn[b, h, 0:128, 0:128] = pr[qb, h][:, 128:]
                else:
                    attn[b, h, q0:q0 + 128, q0 - 128:q0 + 128] = pr[qb, h]
    return out, attn


# revision 58
# speedup vs baseline: 1.0032x; 1.0032x over previous
"""Block-sparse causal MHA with ALiBi on 8 Trainium2 NeuronCores.

Problem: B=2, T=2048, D=1024, H=16, hd=64. Mask analysis: with
BLOCK_SPARSE_SIZE=128 and WINDOW_SIZE=256, each 128-query block attends
causally to itself and fully to the previous 128-block (window constraint is
implied: max dist = 255). Sharding: core i = (batch i//4, seq chunk i%4 of 512
queries). Each core computes QKV projections from a host-pre-transposed x
slice (640 kv rows = 512 own + 128 prev), block-sparse attention for all 16
heads, and the output projection for its 512 rows. Host assembles the full
(out, attn) tuple; attn is zero outside the block-sparse bands.

Softmax is computed without a max-reduction: scores are shifted by the
compile-time row bound C_h[q] = slope_h*(128+qi) + 20 (>= row max up to the
bounded QK term), applied via the ACT exp bias. The ALiBi+mask table B_h
(exactly representable in bf16: integer dists times power-of-2 slopes) is
accumulated into the score PSUM via an identity-weighted matmul.
"""

import math
import sys

sys.path.insert(0, "/opt/trn_rl_repo")

import numpy as np

import concourse.bass as bass
import concourse.mybir as mybir
import concourse.tile as tile
from concourse import bacc, bass_utils
from concourse.masks import make_identity

# ---- problem constants (hardcoded; kernel.py must be self-contained) ----
B = 2
T = 2048
DM = 1024
H = 16
HD = 64
NCORES = 8
SCHUNK = 4          # seq chunks per batch
TQ = T // SCHUNK    # queries per core (512)
PREV = 128          # previous block carried for K/V
TKV = TQ + PREV     # kv rows per core (640)
QB = TQ // 128      # 128-query blocks per core (4)
KB = TKV // 128     # 128-kv blocks per core (5)
NK = DM // 128      # contraction chunks (8)
MASKVAL = -1e9
SCALE = 1.0 / math.sqrt(HD)
CPAD = 20.0         # safety margin over the QK-score magnitude

F32 = mybir.dt.float32
BF16 = mybir.dt.bfloat16
XW_DT = BF16        # dtype for x/W matmul operands
AV_DT = BF16        # dtype for P^T and V in the attn@V matmul
P_DT = F32          # probs staging dtype

_SLOPES = [2.0 ** (-8.0 * (h + 1) / H) for h in range(H)]


def build_program():
    nc = bacc.Bacc("TRN2", target_bir_lowering=False, debug=False)

    xt_d = nc.dram_tensor("xt", (DM, TKV), XW_DT, kind="ExternalInput")
    wq_d = nc.dram_tensor("wq", (DM, DM), XW_DT, kind="ExternalInput")
    wk_d = nc.dram_tensor("wk", (DM, DM), XW_DT, kind="ExternalInput")
    wv_d = nc.dram_tensor("wv", (DM, DM), XW_DT, kind="ExternalInput")
    wo_d = nc.dram_tensor("wo", (DM, DM), XW_DT, kind="ExternalInput")
    bqs_d = nc.dram_tensor("bqs", (DM,), F32, kind="ExternalInput")  # 0.125*b_q
    bk_d = nc.dram_tensor("bk", (DM,), F32, kind="ExternalInput")
    co_d = nc.dram_tensor("co", (DM,), F32, kind="ExternalInput")   # b_v@W_o+b_o
    cb_d = nc.dram_tensor("cb", (128, 2 * H), F32, kind="ExternalInput")  # -C_h[q]
    # ALiBi+mask tables: hi = bf16(slope*dist(+mask)), lo = bf16 residual of
    # the unmasked bias (needed only for the 8 irrational-slope even heads)
    bm_d = nc.dram_tensor("bm", (H, 128, 256), BF16, kind="ExternalInput")
    # per-core qb0 prev-half mask correction: -1e9 on first-chunk cores, 0 else
    mc_d = nc.dram_tensor("mc", (128, 128), BF16, kind="ExternalInput")
    blo_d = nc.dram_tensor("blo", (H // 2, 128, 256), BF16, kind="ExternalInput")

    y_d = nc.dram_tensor("y", (TQ, DM), F32, kind="ExternalOutput")
    probs_d = nc.dram_tensor("probs", (QB * H, 128, 256), P_DT,
                             kind="ExternalOutput")

    with tile.TileContext(nc) as tc:
        with (
            tc.tile_pool(name="persist", bufs=1) as persist,
            tc.tile_pool(name="attn_sb", bufs=6) as attn_sb,
            tc.tile_pool(name="pstage", bufs=2) as pstage_pool,
            tc.tile_pool(name="small", bufs=8) as small,
            tc.tile_pool(name="proj_ps", bufs=2, space="PSUM") as proj_ps,
            tc.tile_pool(name="s_ps", bufs=2, space="PSUM") as s_ps,
            tc.tile_pool(name="pt_ps", bufs=2, space="PSUM") as pt_ps,
            tc.tile_pool(name="ot_ps", bufs=2, space="PSUM") as ot_ps,
        ):
            # ---- PE warmup: matmuls on a zeroed tile keep the HAM clock
            # gate open while the input DMAs stream in ----
            warm = persist.tile([128, 512], BF16, tag="warm", name="warm")
            nc.vector.memset(warm, 0.0)
            for w in range(40):
                wps = proj_ps.tile([128, TQ], F32, tag="proj", name="wps")
                nc.tensor.matmul(wps, warm[:, 0:128], warm, start=True, stop=True)

            # ---- bulk input loads: one DMA per tensor, split across the two
            # HWDGE queues (SP and ACT) ----
            xt = persist.tile([128, NK * TKV], XW_DT, tag="xt", name="xt")

            def load_w(handle, qeng, name):
                t = persist.tile([128, NK * DM], XW_DT, tag=name, name=name)
                qeng.dma_start(
                    out=t.rearrange("p (k j) -> p k j", j=DM),
                    in_=handle[:, :].rearrange("(k p) j -> p k j", p=128))
                return t

            # xt and wq land first, split into k-halves interleaved across
            # the two HWDGE queues so the first Qt matmuls start early;
            # wk/wv follow, wo (needed last) trails
            wq = persist.tile([128, NK * DM], XW_DT, tag="wq_sb", name="wq_sb")
            for kh in range(2):
                k0, k1 = kh * (NK // 2), (kh + 1) * (NK // 2)
                nc.sync.dma_start(
                    out=xt.rearrange("p (k j) -> p k j", j=TKV)[:, k0:k1, :],
                    in_=xt_d[k0 * 128:k1 * 128, :]
                    .rearrange("(k p) j -> p k j", p=128))
                nc.scalar.dma_start(
                    out=wq.rearrange("p (k j) -> p k j", j=DM)[:, k0:k1, :],
                    in_=wq_d[k0 * 128:k1 * 128, :]
                    .rearrange("(k p) j -> p k j", p=128))
            wk = load_w(wk_d, nc.sync, "wk_sb")
            wv = load_w(wv_d, nc.sync, "wv_sb")
            wo = load_w(wo_d, nc.sync, "wo_sb")

            bq_sb = persist.tile([128, NK], F32, tag="bq_sb", name="bq_sb")
            bk_sb = persist.tile([128, NK], F32, tag="bk_sb", name="bk_sb")
            nc.gpsimd.dma_start(out=bq_sb,
                                in_=bqs_d[:].rearrange("(c p) -> p c", p=128))
            nc.gpsimd.dma_start(out=bk_sb,
                                in_=bk_d[:].rearrange("(c p) -> p c", p=128))

            cb_sb = persist.tile([128, 2 * H], F32, tag="cb_sb", name="cb_sb")
            nc.gpsimd.dma_start(out=cb_sb, in_=cb_d[:, :])

            co_sb = persist.tile([128, DM], F32, tag="co_sb", name="co_sb")
            co_ap = co_d[:]
            co_bcast = bass.AP(
                tensor=co_ap.tensor, offset=co_ap.offset,
                ap=[[0, 128]] + list(co_ap.ap),
            )
            nc.gpsimd.dma_start(out=co_sb, in_=co_bcast)

            bm_sb = persist.tile([128, H * 256], BF16, tag="bm_sb", name="bm_sb")
            mc_sb = persist.tile([128, 128], BF16, tag="mc_sb", name="mc_sb")
            blo_sb = persist.tile([128, (H // 2) * 256], BF16, tag="blo_sb",
                                  name="blo_sb")
            nc.sync.dma_start(
                out=bm_sb.rearrange("p (h j) -> p h j", j=256),
                in_=bm_d[:, :, :].rearrange("h p j -> p h j"))
            nc.gpsimd.dma_start(out=mc_sb, in_=mc_d[:, :])
            nc.sync.dma_start(
                out=blo_sb.rearrange("p (h j) -> p h j", j=256),
                in_=blo_d[:, :, :].rearrange("h p j -> p h j"))

            ident = persist.tile([128, 128], F32, tag="ident", name="ident")
            make_identity(nc, ident)
            ident_bf = persist.tile([128, 128], BF16, tag="ident_bf",
                                    name="ident_bf")
            make_identity(nc, ident_bf)


            # ---- projections ----
            # Qt[c] = ((x @ W_q)*0.125 + 0.125*b_q)^T col-block: [128, 512]
            qt = [persist.tile([128, TQ], XW_DT, tag=f"qt{c}", name=f"qt{c}")
                  for c in range(NK)]
            kt = [persist.tile([128, TKV], XW_DT, tag=f"kt{c}", name=f"kt{c}")
                  for c in range(NK)]
            for c in range(NK):
                ps = proj_ps.tile([128, TQ], F32, tag="proj", name="qt_ps")
                for k in range(NK):
                    nc.tensor.matmul(
                        ps,
                        wq[:, k * DM + c * 128:k * DM + (c + 1) * 128],
                        xt[:, k * TKV + PREV:(k + 1) * TKV],
                        start=(k == 0), stop=(k == NK - 1))
                if c % 2 == 0:
                    nc.vector.tensor_scalar(
                        out=qt[c], in0=ps, scalar1=SCALE,
                        scalar2=bq_sb[:, c:c + 1],
                        op0=mybir.AluOpType.mult, op1=mybir.AluOpType.add)
                else:
                    nc.scalar.activation(
                        out=qt[c], in_=ps,
                        func=mybir.ActivationFunctionType.Identity,
                        bias=bq_sb[:, c:c + 1], scale=SCALE)

                for (n0, n1) in ((0, TQ), (TQ, TKV)):
                    ps2 = proj_ps.tile([128, TQ], F32, tag="proj", name="kt_ps")
                    for k in range(NK):
                        nc.tensor.matmul(
                            ps2[:, 0:n1 - n0],
                            wk[:, k * DM + c * 128:k * DM + (c + 1) * 128],
                            xt[:, k * TKV + n0:k * TKV + n1],
                            start=(k == 0), stop=(k == NK - 1))
                    if c % 2 == 0:
                        nc.vector.tensor_scalar(
                            out=kt[c][:, n0:n1], in0=ps2[:, 0:n1 - n0],
                            scalar1=1.0, scalar2=bk_sb[:, c:c + 1],
                            op0=mybir.AluOpType.mult, op1=mybir.AluOpType.add)
                    else:
                        nc.scalar.activation(
                            out=kt[c][:, n0:n1], in_=ps2[:, 0:n1 - n0],
                            func=mybir.ActivationFunctionType.Identity,
                            bias=bk_sb[:, c:c + 1], scale=1.0)

            # V natural layout: 5 tiles [128 kv rows, 1024 cols]
            vt = [persist.tile([128, DM], AV_DT, tag=f"vt{t}", name=f"vt{t}")
                  for t in range(KB)]

            def emit_v(t):
                for half in range(2):
                    ps = proj_ps.tile([128, TQ], F32, tag="proj", name="v_ps")
                    for k in range(NK):
                        nc.tensor.matmul(
                            ps,
                            xt[:, k * TKV + t * 128:k * TKV + (t + 1) * 128],
                            wv[:, k * DM + half * 512:k * DM + (half + 1) * 512],
                            start=(k == 0), stop=(k == NK - 1))
                    nc.vector.tensor_copy(
                        out=vt[t][:, half * 512:(half + 1) * 512], in_=ps)

            with tc.high_priority(offset=-1000):
                for t in range(KB):
                    emit_v(t)

            # ---- attention + output projection, per query block ----
            ot_sb = [[attn_sb.tile([128, 128], AV_DT, tag=f"ot{qb}_{p}",
                                   name=f"ot{qb}_{p}", bufs=1)
                      for p in range(H // 2)] for qb in range(QB)]
            for qb in range(QB):
                pst = pstage_pool.tile([128, H * 256], P_DT, tag="pst", name="pst")
                for h in range(H):
                    c, hi = h // 2, h % 2
                    # scores S = Qt_h^T @ Kt_h + B_h  [128q, 256k] in PSUM
                    s_psum = s_ps.tile([128, 256], F32, tag="s", name="s_psum")
                    nc.tensor.matmul(
                        s_psum,
                        qt[c][hi * 64:(hi + 1) * 64, qb * 128:(qb + 1) * 128],
                        kt[c][hi * 64:(hi + 1) * 64, qb * 128:qb * 128 + 256],
                        start=True, stop=False)
                    even = (h % 2 == 0)
                    last_fold = 1 + (1 if even else 0) + (1 if qb == 0 else 0)
                    fold_i = 1
                    nc.tensor.matmul(
                        s_psum, ident_bf, bm_sb[:, h * 256:(h + 1) * 256],
                        start=False, stop=(fold_i == last_fold))
                    if even:
                        fold_i += 1
                        nc.tensor.matmul(
                            s_psum, ident_bf,
                            blo_sb[:, (h // 2) * 256:(h // 2 + 1) * 256],
                            start=False, stop=(fold_i == last_fold))
                    if qb == 0:
                        fold_i += 1
                        nc.tensor.matmul(
                            s_psum[:, 0:128], ident_bf, mc_sb,
                            start=False, stop=(fold_i == last_fold))
                    # p_raw = exp(S + B - C_h), rowsum accumulated per query
                    p_raw = attn_sb.tile([128, 256], F32, tag="p_raw",
                                         name="p_raw")
                    sumexp = small.tile([128, 1], F32, tag="sumexp", name="sumexp")
                    nc.scalar.activation(
                        out=p_raw, in_=s_psum,
                        func=mybir.ActivationFunctionType.Exp,
                        bias=cb_sb[:, (H + h if qb == 0 else h):
                                    (H + h if qb == 0 else h) + 1],
                        scale=1.0, accum_out=sumexp)
                    rinv = small.tile([128, 1], F32, tag="rinv", name="rinv")
                    nc.vector.reciprocal(rinv, sumexp)
                    # normalized probs (also the DMA staging slice), on GPSIMD
                    pn = pst[:, h * 256:(h + 1) * 256]
                    nc.gpsimd.tensor_scalar_mul(pn, p_raw, rinv)

                    # bf16 copy of normalized probs feeds the transposes
                    pnb = attn_sb.tile([128, 256], BF16, tag="pnb", name="pnb")
                    nc.vector.tensor_copy(out=pnb, in_=pn)
                    ptp = pt_ps.tile([128, 256], BF16, tag="pt", name="ptp")
                    nc.tensor.transpose(ptp[:, 0:128], pnb[:, 0:128], ident_bf)
                    nc.tensor.transpose(ptp[:, 128:256], pnb[:, 128:256],
                                        ident_bf)
                    pt_sb = attn_sb.tile([128, 256], AV_DT, tag="pt_sb",
                                         name="pt_sb")
                    nc.vector.tensor_copy(out=pt_sb, in_=ptp)

                    # attn-out^T [64, 128q], accumulated over 2 key chunks
                    otp = ot_ps.tile([64, 128], F32, tag="ot", name="otp")
                    for kc in range(2):
                        nc.tensor.matmul(
                            otp, vt[qb + kc][:, h * 64:(h + 1) * 64],
                            pt_sb[:, kc * 128:(kc + 1) * 128],
                            start=(kc == 0), stop=(kc == 1))
                    if h % 2 == 0:
                        nc.vector.tensor_copy(
                            out=ot_sb[qb][c][hi * 64:(hi + 1) * 64, :], in_=otp)
                    else:
                        nc.scalar.copy(
                            out=ot_sb[qb][c][hi * 64:(hi + 1) * 64, :], in_=otp)
                    if qb == QB - 1 and h in (3, 7, 11):
                        h0 = h - 3
                        nc.sync.dma_start(
                            out=probs_d[qb * H + h0:qb * H + h + 1, :, :]
                            .rearrange("h p j -> p h j"),
                            in_=pst[:, h0 * 256:(h + 1) * 256]
                            .rearrange("p (h j) -> p h j", j=256))



                if qb == QB - 1:
                    nc.sync.dma_start(
                        out=probs_d[qb * H + 12:(qb + 1) * H, :, :]
                        .rearrange("h p j -> p h j"),
                        in_=pst[:, 12 * 256:].rearrange("p (h j) -> p h j", j=256))
                else:
                    nc.sync.dma_start(
                        out=probs_d[qb * H:(qb + 1) * H, :, :]
                        .rearrange("h p j -> p h j"),
                        in_=pst.rearrange("p (h j) -> p h j", j=256))

                # output projection for this query block
                y_sb = attn_sb.tile([128, DM], F32, tag="y_sb", name="y_sb", bufs=2)
                wo_prio = tc.high_priority(offset=-1000 if qb < QB - 1 else 0)
                with wo_prio:
                 for half in range(2):
                    yp = proj_ps.tile([128, TQ], F32, tag="proj", name="yp")
                    for p in range(NK):
                        nc.tensor.matmul(
                            yp, ot_sb[qb][p],
                            wo[:, p * DM + half * 512:p * DM + (half + 1) * 512],
                            start=(p == 0), stop=(p == NK - 1))
                    nc.vector.tensor_add(
                        y_sb[:, half * 512:(half + 1) * 512], yp,
                        co_sb[:, half * 512:(half + 1) * 512])
                    nc.sync.dma_start(
                        out=y_d[qb * 128:(qb + 1) * 128,
                                half * 512:(half + 1) * 512],
                        in_=y_sb[:, half * 512:(half + 1) * 512])

    nc.compile()
    return nc


_NC = None


def _get_program():
    global _NC
    if _NC is None:
        _NC = build_program()
    return _NC


def _np_dt(dt):
    return mybir.dt.np(dt)


def _host_tables():
    import ml_dtypes
    bf16 = ml_dtypes.bfloat16
    qi = np.arange(128, dtype=np.float32)[:, None]
    kj = np.arange(256, dtype=np.float32)[None, :]
    left = 128.0 + qi - kj
    right = qi - (kj - 128.0)
    dist = np.where(kj < 128, left, right).astype(np.float32)  # unmasked
    dm = np.where(kj < 128, left,
                  np.where(right >= 0, right, MASKVAL)).astype(np.float32)
    dmf = dm.copy()
    dmf[:, :128] = MASKVAL
    slopes = np.asarray(_SLOPES, np.float32)
    bm = np.stack([(s * dm) for s in slopes]).astype(bf16)          # (16,128,256)
    blo = np.stack([
        (slopes[h] * dist) - bm[h].astype(np.float32)
        for h in range(0, H, 2)
    ]).astype(np.float32)
    # residual only where unmasked (masked entries are hugely negative anyway)
    blo[:, dm == np.float32(MASKVAL)] = 0.0
    blo = blo.astype(bf16)
    cb = -(slopes[None, :] * (128.0 + qi) + CPAD).astype(np.float32)
    # qb==0 variant: on first-chunk cores the prev block is masked, so the
    # row max bound is slope*qi, not slope*(128+qi)
    cb0 = -(slopes[None, :] * qi + CPAD).astype(np.float32)
    cb_first = np.concatenate([cb, cb0], axis=1)
    cb_rest = np.concatenate([cb, cb], axis=1)
    mc_first = np.full((128, 128), MASKVAL, np.float32).astype(bf16)
    mc_rest = np.zeros((128, 128), np.float32).astype(bf16)
    return bm, blo, (np.ascontiguousarray(cb_first, dtype=np.float32),
                     np.ascontiguousarray(cb_rest, dtype=np.float32)), (
                         mc_first, mc_rest)


def kernel(x, W_q, b_q, W_k, b_k, W_v, b_v, W_o, b_o):
    x = np.asarray(x, np.float32)
    xw_np = _np_dt(XW_DT)

    nc = _get_program()

    wq_c = np.asarray(W_q, np.float32).astype(xw_np)
    wk_c = np.asarray(W_k, np.float32).astype(xw_np)
    wv_c = np.asarray(W_v, np.float32).astype(xw_np)
    wo_c = np.asarray(W_o, np.float32).astype(xw_np)
    bqs = (np.asarray(b_q, np.float32) * SCALE).astype(np.float32)
    bk = np.asarray(b_k, np.float32)
    co = (np.asarray(b_v, np.float64) @ np.asarray(W_o, np.float64)
          + np.asarray(b_o, np.float64)).astype(np.float32)
    bm, blo, (cb_first, cb_rest), (mc_first, mc_rest) = _host_tables()

    in_maps = []
    for i in range(NCORES):
        b, s = i // SCHUNK, i % SCHUNK
        if s == 0:
            xs = np.concatenate(
                [np.zeros((PREV, DM), np.float32), x[b, 0:TQ]], axis=0)
        else:
            xs = x[b, s * TQ - PREV:(s + 1) * TQ]
        xT = np.ascontiguousarray(xs.T).astype(xw_np)
        in_maps.append({
            "xt": xT, "wq": wq_c, "wk": wk_c, "wv": wv_c, "wo": wo_c,
            "bqs": bqs, "bk": bk, "co": co,
            "cb": cb_first if s == 0 else cb_rest,
            "bm": bm, "mc": mc_first if s == 0 else mc_rest, "blo": blo,
        })

    res = bass_utils.run_bass_kernel_spmd(nc, in_maps,
                                          core_ids=list(range(NCORES)))

    out = np.empty((B, T, DM), np.float32)
    attn = np.zeros((B, H, T, T), np.float32)
    for i in range(NCORES):
        b, s = i // SCHUNK, i % SCHUNK
        out[b, s * TQ:(s + 1) * TQ] = res.results[i]["y"]
        pr = res.results[i]["probs"].astype(np.float32).reshape(QB, H, 128, 256)
        for qb in range(QB):
            g = SCHUNK * s + qb
            q0 = 128 * g
            for h in range(H):
                if g == 0:
                    attn[b, h, 0:128, 0:128] = pr[qb, h][:, 128:]
                else:
                    attn[b, h, q0:q0 + 128, q0 - 128:q0 + 128] = pr[qb, h]
    return out, attn
